# revision 62
# baseline (speedup 1.0000x reference)
"""Trainium2 Bass kernel for GQA attention with RoPE (nn_Attention_21603685499660).

Shapes (hardcoded): x [2, 2048, 4096], H=32 Q heads, KVH=8 KV heads, HD=128.
Sharding over 8 NeuronCores: core c -> batch b = c//4, head-group g = c%4
(8 Q heads, 2 KV heads per core).  Each core computes a partial output
(its heads' attention output through its slice of wo); the host sums the
4 partials per batch.  No on-device collectives.

Per-core pipeline (all matmuls bf16 with f32 PSUM accumulation):
  1. QKV projection from host-pre-transposed x and weights.  Q/K are
     produced directly in transposed [HD, seq] layout; V in natural
     [seq, HD] layout.  RoPE applied entirely on DVE: head dims are
     de-interleaved host-side so the rotation is a half-swap, the sign
     is folded into the prepared sin table ([-sin; +sin]), and the
     half-swap itself rides the sin-multiply via 64-partition-shifted
     access patterns (no TensorE involvement).
  2. Attention with scores computed transposed: ST[k,q] = K @ Q^T per
     (head, q piece, 128-wide k tile).  Softmax without max subtraction
     (scores are O(+-10); exp is safe in f32): P = exp(ST), applied mask
     is multiplicative (exp(mask), 0/1 for causal) on the P tile in
     bf16.  On diagonal tiles QK + exp run only on the live (unmasked)
     q sub-range.  The denominator l is quad-reduced on DVE and
     accumulated ALREADY BROADCAST on TensorE (ones128^T @ quad into a
     PSUM bank); the head output is evacuated+normalized in one DVE
     tensor_mul against a fast-reciprocal of that broadcast (no ScalarE
     Ln/Exp, so ScalarE runs exp-only with no activation-table thrash).
     Causal masks (detected host-side) skip fully-masked k tiles.  The
     q dimension is processed in pieces [512,512,512,256,256]; the
     narrow tail pieces pair adjacent k-tiles into one PSUM bank with a
     single exp per pair (halving ScalarE fixed overhead where it paces)
     and let the final piece's output projection overlap.
  3. Output projection po[q,n] += attnT[d,q]^T @ woT[d,n], emitted as
     PE filler interleaved into subsequent pieces' attention (covers
     the ACT-latency bubbles), with piece p's groups running during
     p+1; output DMAs alternate HW-DGE queues in the drain, and the
     partial outputs ship bf16 (summed f32 on host) to halve the
     33.5MB/core output traffic.

  Clock caveat: the chip's power governor sometimes latches the PE at
  2.0GHz (P0) for a whole run instead of 2.4GHz — a per-run lottery
  worth ~150us.  Denser early-phase schedules (e.g. bulk constants on
  the Act HW-DGE queue, or untapered q pieces) measured as latched far
  more often; the current schedule keeps a brief natural stall at
  ~30us and measured 747-758us across good draws.
"""

from contextlib import ExitStack

import numpy as np
import ml_dtypes

import concourse.bass as bass
import concourse.tile as tile
from concourse import bacc, mybir
from concourse.bass_utils import run_bass_kernel_spmd

B, S, D = 2, 2048, 4096
H, KVH, HD = 32, 8, 128
N_CORES = 8
GROUPS = 4            # head groups (tensor-parallel dim); B * GROUPS = 8 cores
HL = H // GROUPS      # 8 local Q heads
KVL = KVH // GROUPS   # 2 local KV heads
FQK = HL + KVL        # 10 feature tiles of 128 (Q heads then K heads)
NJ = S // 512         # 4 seq chunks of 512 (stage-1 granularity)
NT = S // 128         # 16 seq tiles of 128
ND = D // 128         # 32 contraction tiles
BF = mybir.dt.bfloat16
F32 = mybir.dt.float32

# attention q pieces (start, width); tapered tail so the last pieces'
# output projection can overlap preceding pieces
PIECES = [(0, 512), (512, 512), (1024, 512), (1536, 256), (1792, 256)]

_BUILD_CACHE: dict = {}


def _build(mask_mode: str):
    """mask_mode: 'causal' | 'zero' | 'general'."""
    nc = bacc.Bacc("TRN2", target_bir_lowering=False, debug=False,
                   num_devices=N_CORES)

    xt_d = nc.dram_tensor("xt", [128, ND, S], BF, kind="ExternalInput").ap()
    wqk_d = nc.dram_tensor("wqk", [FQK, 128, ND, 128], BF, kind="ExternalInput").ap()
    wv_d = nc.dram_tensor("wv", [128, ND, KVL * HD], BF, kind="ExternalInput").ap()
    wo_d = nc.dram_tensor("wo", [128, HL, D], BF, kind="ExternalInput").ap()
    cos_d = nc.dram_tensor("cosd", [128, S], F32, kind="ExternalInput").ap()
    sin_d = nc.dram_tensor("sind", [128, S], F32, kind="ExternalInput").ap()
    if mask_mode == "causal":
        mk_d = nc.dram_tensor("maskd", [NJ, 4, 128, 512], BF, kind="ExternalInput").ap()
    elif mask_mode == "general":
        mk_d = nc.dram_tensor("maskt", [S, S], BF, kind="ExternalInput").ap()
    # partial outputs ship bf16 (host sums in f32): halves the 33.5MB/core
    # output DMA, shrinking the drain backlog and post-matmul flush; the
    # ~0.4% partial-sum rounding is small against the 2e-2 budget
    po_d = nc.dram_tensor("po", [S, D], BF, kind="ExternalOutput").ap()

    with tile.TileContext(nc) as tc, ExitStack() as ctx:
        resident = ctx.enter_context(tc.tile_pool(name="resident", bufs=1))
        qkv = ctx.enter_context(tc.tile_pool(name="qkv", bufs=1))

        ones128 = resident.tile([128, 128], BF)
        nc.vector.memset(ones128[:], 1.0)

        QT = qkv.tile([128, HL, S], BF)    # [HD, head, seq] (de-interleaved rows)
        KT = qkv.tile([128, KVL, S], BF)
        V = qkv.tile([128, NT, KVL * HD], BF)  # [seq%128, seqtile, kv-head*HD]

        # ---- stage 1: QKV projection + RoPE ----
        with tc.tile_pool(name="s1const", bufs=1) as s1const, \
             tc.tile_pool(name="xpool", bufs=2) as xpool, \
             tc.tile_pool(name="wpool", bufs=3) as wpool, \
             tc.tile_pool(name="tpool", bufs=3) as tpool, \
             tc.tile_pool(name="ps_qk", bufs=3, space="PSUM") as ps_qk, \
             tc.tile_pool(name="ps_w", bufs=2, space="PSUM") as ps_w, \
             tc.tile_pool(name="ps_v", bufs=2, space="PSUM") as ps_v:
            cosb = s1const.tile([128, S], F32)
            sinb = s1const.tile([128, S], F32)  # [-sin; +sin] halves
            wvb = s1const.tile([128, ND, KVL * HD], BF)
            # PE warm-up: dense ones@ones matmuls (no DMA dependency) keep
            # TensorE busy through the HAM window while the first x/weight
            # DMAs land, so real matmuls start at full clock.
            for _ in range(72):
                wtile = ps_w.tile([128, 128], F32, tag="warm")
                nc.tensor.matmul(wtile[:], ones128[:], ones128[:],
                                 start=True, stop=True)

            def rope_emit(raw, f, js):
                # o = raw*cos + halfswap(raw)*sinN with no TensorE: the
                # half-swap is two partition-shifted ScalarE copies (same
                # engine as the evacuation, so ordering is free) and the
                # rotation sign lives in sinb = [-sin; +sin].
                rot = tpool.tile([128, 512], BF, tag="rot")
                nc.scalar.copy(out=rot[0:64, :], in_=raw[64:128, :])
                nc.scalar.copy(out=rot[64:128, :], in_=raw[0:64, :])
                t1 = tpool.tile([128, 512], F32, tag="t1")
                nc.vector.tensor_mul(t1[:], raw[:], cosb[:, js])
                t2 = tpool.tile([128, 512], F32, tag="t2")
                nc.vector.tensor_mul(t2[:], rot[:], sinb[:, js])
                dest = QT[:, f, js] if f < HL else KT[:, f - HL, js]
                nc.vector.tensor_add(dest, t1[:], t2[:])

            # weight prefetch runs two groups deep so the next chunk's 4MB
            # x DMA (queued between them on the same SP HW-DGE queue) never
            # head-of-line blocks the first weight tiles of the new chunk
            n_groups = NJ * FQK
            wtiles: dict = {}
            wissued = 0

            def wprefetch(upto):
                nonlocal wissued
                while wissued < min(n_groups, upto):
                    wt = wpool.tile([128, ND, 128], BF, tag="wf")
                    nc.sync.dma_start(out=wt[:], in_=wqk_d[wissued % FQK])
                    wtiles[wissued] = wt
                    wissued += 1

            wprefetch(2)
            for j in range(NJ):
                js = bass.ts(j, 512)
                xj = xpool.tile([128, ND, 512], BF)
                for n in range(ND):
                    nc.sync.dma_start(out=xj[:, n, :], in_=xt_d[:, n, js])
                for f in range(FQK):
                    gi = j * FQK + f
                    wf = wtiles.pop(gi)
                    wprefetch(gi + 3)
                    if f == 0:
                        # cos/sin loaded per chunk (0.5MB slices) and wv
                        # deferred to mid-chunk-0: keeps the SP queue free
                        # of multi-MB head-of-line insertions (all stay on
                        # the SP queue — moving them to the Act queue
                        # measured 937us, likely P0-governor related)
                        nc.sync.dma_start(out=cosb[:, js], in_=cos_d[:, js])
                        nc.sync.dma_start(out=sinb[:, js], in_=sin_d[:, js])
                    if j == 0 and f == 8:
                        nc.sync.dma_start(out=wvb[:], in_=wv_d[:])
                    ps = ps_qk.tile([128, 512], F32, tag="qk")
                    for n in range(ND):
                        nc.tensor.matmul(ps[:], wf[:, n, :], xj[:, n, :],
                                         start=(n == 0), stop=(n == ND - 1))
                    raw = tpool.tile([128, 512], BF, tag="raw")
                    nc.scalar.copy(out=raw[:], in_=ps[:])
                    rope_emit(raw, f, js)
                for tt in range(4):
                    psv = ps_v.tile([128, KVL * HD], F32, tag="v")
                    for n in range(ND):
                        nc.tensor.matmul(psv[:], xj[:, n, bass.ts(tt, 128)],
                                         wvb[:, n, :],
                                         start=(n == 0), stop=(n == ND - 1))
                    nc.scalar.copy(out=V[:, j * 4 + tt, :], in_=psv[:])

        # attnT + wo live from stage 2 through stage 3 (pool opened only now
        # so stage 1 had the SBUF).
        att_out = ctx.enter_context(tc.tile_pool(name="att_out", bufs=1))
        attnT = att_out.tile([128, HL, S], BF)  # [HD, head, seq]
        wob = att_out.tile([128, HL, D], BF)

        # ---- stage 2+3: attention with interleaved output projection ----
        # Per (piece, h) k-loop: QK -> exp -> (0/1 mask multiply in bf16
        # SBUF) -> [l, PV] where the softmax denominator accumulates
        # broadcast on TensorE (ones128^T @ quad into a PSUM bank) so no
        # serial DVE chain gates the pipeline; 1/l is one fast-reciprocal
        # DVE op.  The raw output is evacuated and normalized on DVE.
        # Output-projection (po) matmul groups for piece p are emitted
        # during later pieces' head loops — dense PE filler for the
        # ACT-bound attention stretches.
        po_state = {"cur": None, "dd": 0, "drain": False, "alt": False}

        def po_step(budget):
            # emit up to `budget` output-projection matmuls as PE filler;
            # a group's PSUM accumulation legally interleaves with other
            # banks' matmuls, so groups can be spread across many call sites
            for _ in range(budget):
                if po_state["cur"] is None:
                    if not pending_po:
                        return
                    qt, nn = pending_po.pop(0)
                    pop = ps_po.tile([128, 512], F32, tag="po")
                    po_state["cur"] = (qt, nn, pop)
                    po_state["dd"] = 0
                qt, nn, pop = po_state["cur"]
                dd = po_state["dd"]
                nc.tensor.matmul(pop[:], attnT[:, dd, bass.ts(qt, 128)],
                                 wob[:, dd, bass.ts(nn, 512)],
                                 start=(dd == 0), stop=(dd == HL - 1))
                po_state["dd"] += 1
                if po_state["dd"] == HL:
                    stg = spool.tile([128, 512], BF, tag="stg")
                    nc.vector.tensor_copy(stg[:], pop[:])
                    # in the final drain ScalarE is idle: alternate output
                    # DMAs onto its HW-DGE queue so the write backlog
                    # doesn't tail the last matmuls on a single queue
                    eng = nc.scalar if (po_state["drain"] and po_state["alt"]) \
                        else nc.sync
                    po_state["alt"] = not po_state["alt"]
                    eng.dma_start(
                        out=po_d[bass.ts(qt, 128), bass.ts(nn, 512)], in_=stg[:])
                    po_state["cur"] = None

        with tc.tile_pool(name="mpool", bufs=2 if mask_mode != "general" else 1) as mpool, \
             tc.tile_pool(name="ppool", bufs=12) as ppool, \
             tc.tile_pool(name="qpool", bufs=2) as qpool, \
             tc.tile_pool(name="npool", bufs=2) as npool, \
             tc.tile_pool(name="spool", bufs=3) as spool, \
             tc.tile_pool(name="ps_st", bufs=3, space="PSUM") as ps_st, \
             tc.tile_pool(name="ps_o", bufs=1, space="PSUM") as ps_o, \
             tc.tile_pool(name="ps_l", bufs=1, space="PSUM") as ps_l, \
             tc.tile_pool(name="ps_po", bufs=3, space="PSUM") as ps_po:
            pending_po = []  # (qt, nn) groups ready to emit as PE filler
            first_wo = True
            for pidx, (q0, w) in enumerate(PIECES):
                if pidx == len(PIECES) - 1:
                    po_state["drain"] = True
                js = bass.ds(q0, w)
                if mask_mode == "zero":
                    nkt = NT
                    atiles = []
                else:
                    nkt = (q0 + w) // 128 if mask_mode == "causal" else NT
                    if mask_mode == "causal":
                        atiles = list(range(q0 // 128, (q0 + w) // 128))
                    else:
                        atiles = list(range(nkt))
                if atiles:
                    msk = mpool.tile([128, len(atiles), w], BF, tag="msk")
                    for idx, t in enumerate(atiles):
                        if mask_mode == "causal":
                            jj, ii = t // 4, t % 4
                            nc.sync.dma_start(
                                out=msk[:, idx, :],
                                in_=mk_d[jj, ii][:, bass.ds(q0 - 512 * jj, w)])
                        else:
                            nc.sync.dma_start(
                                out=msk[:, idx, :],
                                in_=mk_d[bass.ts(t, 128), js])
                if first_wo:
                    # after the first mask tiles so they aren't queued behind
                    # 8.4MB of wo weights
                    for dd in range(HL):
                        nc.sync.dma_start(out=wob[:, dd, :], in_=wo_d[:, dd, :])
                    first_wo = False

                # l-accumulation groups of 8 k-tiles (one TensorE broadcast
                # matmul per group; the extra DVE adds are cheap vs PE time)
                gend = {}
                tbase = 0
                while tbase < nkt:
                    glen = min(8, nkt - tbase)
                    gend[tbase + glen - 1] = (tbase, glen)
                    tbase += glen

                for h in range(HL):
                    hk = h // (HL // KVL)
                    outp = ps_o.tile([128, w], F32, tag="out")
                    lp = ps_l.tile([128, w], F32, tag="l")
                    pts = []
                    # software pipeline: PV_t is emitted one tile after QK_t so
                    # a full QK + filler sits in the PE stream while exp_t runs.
                    # Diagonal tiles contribute nothing to masked columns, so
                    # PV runs only on the live sub-range (t=0 has off=0 and
                    # start-zeroes the full width; PSUM accum is per-address).
                    def emit_pv(t):
                        off = max(0, 128 * t - q0) if mask_mode == "causal" else 0
                        nc.tensor.matmul(outp[:, off:w], V[:, t, bass.ts(hk, 128)],
                                         pts[t][:, off:w],
                                         start=(t == 0), stop=(t == nkt - 1),
                                         skip_group_check=True)

                    l_started = False
                    n_pv_done = 0
                    # narrow pieces pair adjacent k-tiles into one PSUM bank
                    # and run a single exp over the pair, halving ScalarE's
                    # per-instruction fixed overhead where it paces the tail
                    paired = w <= 256
                    t = 0
                    while t < nkt:
                        npair = 2 if (paired and t + 1 < nkt) else 1
                        stp = ps_st.tile([128, npair, w], F32, tag="st")
                        pt2 = ppool.tile([128, npair, w], BF, tag="pt")
                        for i in range(npair):
                            tt = t + i
                            # causal: columns q < 128t fully masked; compute
                            # QK only on the live sub-range.  Stale (finite)
                            # garbage in dead columns is zeroed by the mask
                            # multiply below.
                            off = (max(0, 128 * tt - q0)
                                   if mask_mode == "causal" else 0)
                            nc.tensor.matmul(stp[:, i, off:w],
                                             KT[:, hk, bass.ts(tt, 128)],
                                             QT[:, h, bass.ds(q0 + off, w - off)],
                                             start=True, stop=True)
                        if npair == 1 and mask_mode == "causal":
                            off = max(0, 128 * t - q0)
                            nc.scalar.activation(
                                out=pt2[:, 0, off:w], in_=stp[:, 0, off:w],
                                func=mybir.ActivationFunctionType.Exp)
                        else:
                            nc.scalar.activation(
                                out=pt2[:], in_=stp[:],
                                func=mybir.ActivationFunctionType.Exp)
                        for i in range(npair):
                            tt = t + i
                            if tt in atiles:
                                # multiplicative mask exp(m): 0/1 for causal;
                                # also zeroes the dead columns
                                nc.vector.tensor_mul(
                                    pt2[:, i, :], pt2[:, i, :],
                                    msk[:, atiles.index(tt), :])
                            pts.append(pt2[:, i, :])
                        # wide pieces meter po filler so backlog survives
                        # into the tapered tail (which otherwise starves and
                        # drops HAM to half clock); narrow pieces drain hard
                        po_step(3 if npair == 2 else 1)
                        # PV lags one tile behind exp so a full QK + filler
                        # sits in the PE stream while exp runs
                        while n_pv_done < len(pts) - 1:
                            emit_pv(n_pv_done)
                            n_pv_done += 1
                        t_last = t + npair - 1
                        t = t_last + 1
                        if t_last in gend:
                            # tree-reduce the group's P tiles on DVE, one
                            # broadcast l matmul per group
                            tb, glen = gend[t_last]
                            grp = pts[tb:tb + glen]
                            if glen == 1:
                                qd = grp[0]
                            else:
                                # pairwise first level, then in-place chain
                                # (bf16 adds of P<=1 values; chain depth <=3)
                                s1 = qpool.tile([128, w], BF, tag="s1")
                                nc.vector.tensor_add(s1[:], grp[0][:], grp[1][:])
                                if len(grp) >= 4:
                                    s2 = qpool.tile([128, w], BF, tag="s2")
                                    nc.vector.tensor_add(s2[:], grp[2][:],
                                                         grp[3][:])
                                    qd = qpool.tile([128, w], BF, tag="qd")
                                    nc.vector.tensor_add(qd[:], s1[:], s2[:])
                                    rest = grp[4:]
                                else:
                                    qd = qpool.tile([128, w], BF, tag="qd")
                                    if len(grp) == 3:
                                        nc.vector.tensor_add(qd[:], s1[:],
                                                             grp[2][:])
                                    else:
                                        nc.vector.tensor_copy(qd[:], s1[:])
                                    rest = []
                                for i in range(0, len(rest) - 1, 2):
                                    sp = qpool.tile([128, w], BF,
                                                    tag=f"sp{i}")
                                    nc.vector.tensor_add(sp[:], rest[i][:],
                                                         rest[i + 1][:])
                                    nc.vector.tensor_add(qd[:], qd[:], sp[:])
                                if len(rest) % 2:
                                    nc.vector.tensor_add(qd[:], qd[:],
                                                         rest[-1][:])
                            nc.tensor.matmul(lp[:], ones128[:], qd[:],
                                             start=not l_started,
                                             stop=(t_last == nkt - 1))
                            l_started = True
                    while n_pv_done < nkt:
                        emit_pv(n_pv_done)
                        n_pv_done += 1
                    # fused evacuation + normalization on DVE (ScalarE stays
                    # exp-only; 1/l is a single fast-reciprocal op on the
                    # TensorE-broadcast denominator)
                    rcp = npool.tile([128, w], F32, tag="rcp")
                    nc.vector.reciprocal_approx_fast(out=rcp[:], in_=lp[:])
                    nc.vector.tensor_mul(attnT[:, h, js], outp[:], rcp[:])
                    # PE filler between heads covers the exp pipeline refill
                    po_step(16)
                pending_po.extend(
                    (qt, nn) for qt in range(q0 // 128, (q0 + w) // 128)
                    for nn in range(D // 512))
            po_state["drain"] = True
            while pending_po or po_state["cur"] is not None:
                po_step(8)

    nc.compile()
    return nc


def _get_nc(mask_mode: str):
    if mask_mode not in _BUILD_CACHE:
        _BUILD_CACHE[mask_mode] = _build(mask_mode)
    return _BUILD_CACHE[mask_mode]


_DEINT = np.concatenate([np.arange(0, HD, 2), np.arange(1, HD, 2)])  # de-interleave


def _host_prep(x, freqs_cos, freqs_sin, mask, wq, wk, wv, wo):
    bf16 = ml_dtypes.bfloat16
    scale = float(HD) ** -0.5

    # mask mode
    mask = np.asarray(mask, np.float32)
    tril = np.tril(np.ones((S, S), bool))
    if np.all(mask == 0):
        mask_mode = "zero"
    elif np.all(mask[tril] == 0) and np.all(mask[~tril] <= -1e8):
        mask_mode = "causal"
    else:
        mask_mode = "general"

    # weights: de-interleave head dims of wq/wk; fold softmax scale into wq
    wq_p = (np.asarray(wq, np.float32).reshape(H, HD, D)[:, _DEINT, :] * scale)
    wk_p = np.asarray(wk, np.float32).reshape(KVH, HD, D)[:, _DEINT, :]
    wv_n = np.asarray(wv, np.float32).reshape(KVH, HD, D)
    wo_n = np.asarray(wo, np.float32)

    per_group = []
    for g in range(GROUPS):
        feats = np.concatenate([
            wq_p[g * HL:(g + 1) * HL].reshape(HL * HD, D),
            wk_p[g * KVL:(g + 1) * KVL].reshape(KVL * HD, D),
        ], axis=0)  # [1280, D]
        wqk_dma = np.ascontiguousarray(
            feats.reshape(FQK, 128, ND, 128).transpose(0, 3, 2, 1)).astype(bf16)
        wvg = wv_n[g * KVL:(g + 1) * KVL].reshape(KVL * HD, D)
        wv_dma = np.ascontiguousarray(
            wvg.reshape(KVL * HD, ND, 128).transpose(2, 1, 0)).astype(bf16)
        woT = wo_n[:, g * HL * HD:(g + 1) * HL * HD].T  # [1024, D]
        wo_dma = np.ascontiguousarray(
            woT.reshape(HL, 128, D).transpose(1, 0, 2)).astype(bf16)
        per_group.append((wqk_dma, wv_dma, wo_dma))

    xs = []
    for b in range(B):
        xT = np.asarray(x[b], np.float32).T  # [D, S]
        xs.append(np.ascontiguousarray(
            xT.reshape(ND, 128, S).transpose(1, 0, 2)).astype(bf16))

    cosT = np.asarray(freqs_cos, np.float32).T  # [64, S]
    sinT = np.asarray(freqs_sin, np.float32).T
    cos_dma = np.ascontiguousarray(np.concatenate([cosT, cosT], 0))
    # rotation sign folded into the sin table: o = raw*cos + halfswap(raw)*sinN
    sin_dma = np.ascontiguousarray(np.concatenate([-sinT, sinT], 0))

    # mask is applied multiplicatively after exp: P *= exp(mask)
    mask_extra = {}
    if mask_mode == "causal":
        mT = np.exp(np.minimum(mask.T, 0.0))
        md = np.empty((NJ, 4, 128, 512), np.float32)
        for j in range(NJ):
            for i in range(4):
                t = 4 * j + i
                md[j, i] = mT[t * 128:(t + 1) * 128, j * 512:(j + 1) * 512]
        mask_extra["maskd"] = md.astype(bf16)
    elif mask_mode == "general":
        with np.errstate(over="ignore"):
            mask_extra["maskt"] = np.ascontiguousarray(
                np.exp(mask.T)).astype(bf16)

    in_maps = []
    for c in range(N_CORES):
        b, g = c // GROUPS, c % GROUPS
        wqk_dma, wv_dma, wo_dma = per_group[g]
        m = {"xt": xs[b], "wqk": wqk_dma, "wv": wv_dma, "wo": wo_dma,
             "cosd": cos_dma, "sind": sin_dma}
        m.update(mask_extra)
        in_maps.append(m)
    return mask_mode, in_maps


def kernel(x, freqs_cos, freqs_sin, positions, mask, wq, wk, wv, wo,
           _want_profile=False):
    mask_mode, in_maps = _host_prep(x, freqs_cos, freqs_sin, mask, wq, wk, wv, wo)
    nc = _get_nc(mask_mode)
    res = run_bass_kernel_spmd(nc, in_maps, core_ids=list(range(N_CORES)),
                               trace=_want_profile)
    out = np.zeros((B, S, D), np.float32)
    for c in range(N_CORES):
        out[c // GROUPS] += np.asarray(res.results[c]["po"], np.float32)
    if _want_profile:
        kernel.last_exec_time_ns = res.exec_time_ns
        kernel.last_results = res
    return out


# revision 63
# speedup vs baseline: 1.1920x; 1.1920x over previous
"""Trainium2 Bass kernel for GQA attention with RoPE (nn_Attention_21603685499660).

Shapes (hardcoded): x [2, 2048, 4096], H=32 Q heads, KVH=8 KV heads, HD=128.
Sharding over 8 NeuronCores: core c -> batch b = c//4, head-group g = c%4
(8 Q heads, 2 KV heads per core).  Each core computes a partial output
(its heads' attention output through its slice of wo); the host sums the
4 partials per batch.  No on-device collectives.

Per-core pipeline (all matmuls bf16 with f32 PSUM accumulation):
  1. QKV projection from host-pre-transposed x and weights.  Q/K are
     produced directly in transposed [HD, seq] layout; V in natural
     [seq, HD] layout.  RoPE applied entirely on DVE: head dims are
     de-interleaved host-side so the rotation is a half-swap, the sign
     is folded into the prepared sin table ([-sin; +sin]), and the
     half-swap itself rides the sin-multiply via 64-partition-shifted
     access patterns (no TensorE involvement).
  2. Attention with scores computed transposed: ST[k,q] = K @ Q^T per
     (head, q piece, 128-wide k tile).  Softmax without max subtraction
     (scores are O(+-10); exp is safe in f32): P = exp(ST), applied mask
     is multiplicative (exp(mask), 0/1 for causal) on the P tile in
     bf16.  On diagonal tiles QK + exp run only on the live (unmasked)
     q sub-range.  The denominator l is quad-reduced on DVE and
     accumulated ALREADY BROADCAST on TensorE (ones128^T @ quad into a
     PSUM bank); the head output is evacuated+normalized in one DVE
     tensor_mul against a fast-reciprocal of that broadcast (no ScalarE
     Ln/Exp, so ScalarE runs exp-only with no activation-table thrash).
     Causal masks (detected host-side) skip fully-masked k tiles.  The
     q dimension is processed in pieces [512,512,512,256,256]; the
     narrow tail pieces pair adjacent k-tiles into one PSUM bank with a
     single exp per pair (halving ScalarE fixed overhead where it paces)
     and let the final piece's output projection overlap.
  3. Output projection po[q,n] += attnT[d,q]^T @ woT[d,n], emitted as
     PE filler interleaved into subsequent pieces' attention (covers
     the ACT-latency bubbles), with piece p's groups running during
     p+1; output DMAs alternate HW-DGE queues in the drain, and the
     partial outputs ship bf16 (summed f32 on host) to halve the
     33.5MB/core output traffic.

  Clock caveat: the chip's power governor sometimes latches the PE at
  2.0GHz (P0) for a whole run instead of 2.4GHz — a per-run lottery
  worth ~150us.  Denser early-phase schedules (e.g. bulk constants on
  the Act HW-DGE queue, or untapered q pieces) measured as latched far
  more often; the current schedule keeps a brief natural stall at
  ~30us and measured 747-758us across good draws.
"""

from contextlib import ExitStack

import numpy as np
import ml_dtypes

import concourse.bass as bass
import concourse.tile as tile
from concourse import bacc, mybir
from concourse.bass_utils import run_bass_kernel_spmd

B, S, D = 2, 2048, 4096
H, KVH, HD = 32, 8, 128
N_CORES = 8
GROUPS = 4            # head groups (tensor-parallel dim); B * GROUPS = 8 cores
HL = H // GROUPS      # 8 local Q heads
KVL = KVH // GROUPS   # 2 local KV heads
FQK = HL + KVL        # 10 feature tiles of 128 (Q heads then K heads)
NJ = S // 512         # 4 seq chunks of 512 (stage-1 granularity)
NT = S // 128         # 16 seq tiles of 128
ND = D // 128         # 32 contraction tiles
BF = mybir.dt.bfloat16
F32 = mybir.dt.float32

# attention q pieces (start, width); tapered tail so the last pieces'
# output projection can overlap preceding pieces
PIECES = [(0, 512), (512, 512), (1024, 512), (1536, 256), (1792, 256)]

_BUILD_CACHE: dict = {}


def _build(mask_mode: str):
    """mask_mode: 'causal' | 'zero' | 'general'."""
    nc = bacc.Bacc("TRN2", target_bir_lowering=False, debug=False,
                   num_devices=N_CORES)

    xt_d = nc.dram_tensor("xt", [128, ND, S], BF, kind="ExternalInput").ap()
    wqk_d = nc.dram_tensor("wqk", [FQK, 128, ND, 128], BF, kind="ExternalInput").ap()
    wv_d = nc.dram_tensor("wv", [128, ND, KVL * HD], BF, kind="ExternalInput").ap()
    wo_d = nc.dram_tensor("wo", [128, HL, D], BF, kind="ExternalInput").ap()
    cos_d = nc.dram_tensor("cosd", [128, S], F32, kind="ExternalInput").ap()
    sin_d = nc.dram_tensor("sind", [128, S], F32, kind="ExternalInput").ap()
    if mask_mode == "causal":
        mk_d = nc.dram_tensor("maskd", [NJ, 4, 128, 512], BF, kind="ExternalInput").ap()
    elif mask_mode == "general":
        mk_d = nc.dram_tensor("maskt", [S, S], BF, kind="ExternalInput").ap()
    # partial outputs ship bf16 (host sums in f32): halves the 33.5MB/core
    # output DMA, shrinking the drain backlog and post-matmul flush; the
    # ~0.4% partial-sum rounding is small against the 2e-2 budget
    po_d = nc.dram_tensor("po", [S, D], BF, kind="ExternalOutput").ap()

    with tile.TileContext(nc) as tc, ExitStack() as ctx:
        resident = ctx.enter_context(tc.tile_pool(name="resident", bufs=1))
        qkv = ctx.enter_context(tc.tile_pool(name="qkv", bufs=1))

        ones128 = resident.tile([128, 128], BF)
        nc.vector.memset(ones128[:], 1.0)

        QT = qkv.tile([128, HL, S], BF)    # [HD, head, seq] (de-interleaved rows)
        KT = qkv.tile([128, KVL, S], BF)
        V = qkv.tile([128, NT, KVL * HD], BF)  # [seq%128, seqtile, kv-head*HD]

        # ---- stage 1: QKV projection + RoPE ----
        with tc.tile_pool(name="s1const", bufs=1) as s1const, \
             tc.tile_pool(name="xpool", bufs=2) as xpool, \
             tc.tile_pool(name="wpool", bufs=3) as wpool, \
             tc.tile_pool(name="tpool", bufs=3) as tpool, \
             tc.tile_pool(name="ps_qk", bufs=2, space="PSUM") as ps_qk, \
             tc.tile_pool(name="ps_w", bufs=2, space="PSUM") as ps_w, \
             tc.tile_pool(name="ps_v", bufs=2, space="PSUM") as ps_v:
            cosb = s1const.tile([128, S], F32)
            sinb = s1const.tile([128, S], F32)  # [-sin; +sin] halves
            wvb = s1const.tile([128, ND, KVL * HD], BF)
            # PE warm-up: dense ones@ones matmuls (no DMA dependency) keep
            # TensorE busy through the HAM window while the first x/weight
            # DMAs land, so real matmuls start at full clock.
            for _ in range(72):
                wtile = ps_w.tile([128, 128], F32, tag="warm")
                nc.tensor.matmul(wtile[:], ones128[:], ones128[:],
                                 start=True, stop=True)

            def rope_emit(raw, f, js):
                # o = raw*cos + halfswap(raw)*sinN with no TensorE: the
                # half-swap is two partition-shifted ScalarE copies (same
                # engine as the evacuation, so ordering is free) and the
                # rotation sign lives in sinb = [-sin; +sin].
                rot = tpool.tile([128, 512], BF, tag="rot")
                nc.scalar.copy(out=rot[0:64, :], in_=raw[64:128, :])
                nc.scalar.copy(out=rot[64:128, :], in_=raw[0:64, :])
                t1 = tpool.tile([128, 512], F32, tag="t1")
                nc.vector.tensor_mul(t1[:], raw[:], cosb[:, js])
                t2 = tpool.tile([128, 512], F32, tag="t2")
                nc.vector.tensor_mul(t2[:], rot[:], sinb[:, js])
                dest = QT[:, f, js] if f < HL else KT[:, f - HL, js]
                nc.vector.tensor_add(dest, t1[:], t2[:])

            # weight prefetch runs two groups deep so the next chunk's 4MB
            # x DMA (queued between them on the same SP HW-DGE queue) never
            # head-of-line blocks the first weight tiles of the new chunk
            n_groups = NJ * FQK
            wtiles: dict = {}
            wissued = 0

            def wprefetch(upto):
                nonlocal wissued
                while wissued < min(n_groups, upto):
                    wt = wpool.tile([128, ND, 128], BF, tag="wf")
                    nc.sync.dma_start(out=wt[:], in_=wqk_d[wissued % FQK])
                    wtiles[wissued] = wt
                    wissued += 1

            wprefetch(2)
            for j in range(NJ):
                js = bass.ts(j, 512)
                xj = xpool.tile([128, ND, 512], BF)
                for n in range(ND):
                    nc.sync.dma_start(out=xj[:, n, :], in_=xt_d[:, n, js])
                for f in range(FQK):
                    gi = j * FQK + f
                    wf = wtiles.pop(gi)
                    wprefetch(gi + 3)
                    if f == 0:
                        # cos/sin loaded per chunk (0.5MB slices) and wv
                        # deferred to mid-chunk-0: keeps the SP queue free
                        # of multi-MB head-of-line insertions (all stay on
                        # the SP queue — moving them to the Act queue
                        # measured 937us, likely P0-governor related)
                        nc.sync.dma_start(out=cosb[:, js], in_=cos_d[:, js])
                        nc.sync.dma_start(out=sinb[:, js], in_=sin_d[:, js])
                    if j == 0 and f == 8:
                        nc.sync.dma_start(out=wvb[:], in_=wv_d[:])
                    ps = ps_qk.tile([128, 512], F32, tag="qk")
                    for n in range(ND):
                        nc.tensor.matmul(ps[:], wf[:, n, :], xj[:, n, :],
                                         start=(n == 0), stop=(n == ND - 1))
                    raw = tpool.tile([128, 512], BF, tag="raw")
                    nc.scalar.copy(out=raw[:], in_=ps[:])
                    rope_emit(raw, f, js)
                for tt in range(4):
                    psv = ps_v.tile([128, KVL * HD], F32, tag="v")
                    for n in range(ND):
                        nc.tensor.matmul(psv[:], xj[:, n, bass.ts(tt, 128)],
                                         wvb[:, n, :],
                                         start=(n == 0), stop=(n == ND - 1))
                    nc.scalar.copy(out=V[:, j * 4 + tt, :], in_=psv[:])

        # attnT + wo live from stage 2 through stage 3 (pool opened only now
        # so stage 1 had the SBUF).
        att_out = ctx.enter_context(tc.tile_pool(name="att_out", bufs=1))
        attnT = att_out.tile([128, HL, S], BF)  # [HD, head, seq]
        wob = att_out.tile([128, HL, D], BF)

        # ---- stage 2+3: attention with interleaved output projection ----
        # Per (piece, h) k-loop: QK -> exp -> (0/1 mask multiply in bf16
        # SBUF) -> [l, PV] where the softmax denominator accumulates
        # broadcast on TensorE (ones128^T @ quad into a PSUM bank) so no
        # serial DVE chain gates the pipeline; 1/l is one fast-reciprocal
        # DVE op.  The raw output is evacuated and normalized on DVE.
        # Output-projection (po) matmul groups for piece p are emitted
        # during later pieces' head loops — dense PE filler for the
        # ACT-bound attention stretches.
        po_state = {"cur": None, "dd": 0, "drain": False, "alt": False}

        def po_step(budget):
            # emit up to `budget` output-projection matmuls as PE filler;
            # a group's PSUM accumulation legally interleaves with other
            # banks' matmuls, so groups can be spread across many call sites
            for _ in range(budget):
                if po_state["cur"] is None:
                    if not pending_po:
                        return
                    qt, nn = pending_po.pop(0)
                    pop = ps_po.tile([128, 512], F32, tag="po")
                    po_state["cur"] = (qt, nn, pop)
                    po_state["dd"] = 0
                qt, nn, pop = po_state["cur"]
                dd = po_state["dd"]
                nc.tensor.matmul(pop[:], attnT[:, dd, bass.ts(qt, 128)],
                                 wob[:, dd, bass.ts(nn, 512)],
                                 start=(dd == 0), stop=(dd == HL - 1))
                po_state["dd"] += 1
                if po_state["dd"] == HL:
                    stg = spool.tile([128, 512], BF, tag="stg")
                    nc.vector.tensor_copy(stg[:], pop[:])
                    # in the final drain ScalarE is idle: alternate output
                    # DMAs onto its HW-DGE queue so the write backlog
                    # doesn't tail the last matmuls on a single queue
                    eng = nc.scalar if (po_state["drain"] and po_state["alt"]) \
                        else nc.sync
                    po_state["alt"] = not po_state["alt"]
                    eng.dma_start(
                        out=po_d[bass.ts(qt, 128), bass.ts(nn, 512)], in_=stg[:])
                    po_state["cur"] = None

        with tc.tile_pool(name="mpool", bufs=2 if mask_mode != "general" else 1) as mpool, \
             tc.tile_pool(name="ppool", bufs=10) as ppool, \
             tc.tile_pool(name="qpool", bufs=2) as qpool, \
             tc.tile_pool(name="npool", bufs=2) as npool, \
             tc.tile_pool(name="spool", bufs=3) as spool, \
             tc.tile_pool(name="ps_st", bufs=3, space="PSUM") as ps_st, \
             tc.tile_pool(name="ps_o", bufs=1, space="PSUM") as ps_o, \
             tc.tile_pool(name="ps_l", bufs=1, space="PSUM") as ps_l, \
             tc.tile_pool(name="ps_po", bufs=3, space="PSUM") as ps_po:
            pending_po = []  # (qt, nn) groups ready to emit as PE filler
            first_wo = True
            for pidx, (q0, w) in enumerate(PIECES):
                if pidx == len(PIECES) - 1:
                    po_state["drain"] = True
                js = bass.ds(q0, w)
                if mask_mode == "zero":
                    nkt = NT
                    atiles = []
                else:
                    nkt = (q0 + w) // 128 if mask_mode == "causal" else NT
                    if mask_mode == "causal":
                        atiles = list(range(q0 // 128, (q0 + w) // 128))
                    else:
                        atiles = list(range(nkt))
                if atiles:
                    msk = mpool.tile([128, len(atiles), w], BF, tag="msk")
                    for idx, t in enumerate(atiles):
                        if mask_mode == "causal":
                            jj, ii = t // 4, t % 4
                            nc.sync.dma_start(
                                out=msk[:, idx, :],
                                in_=mk_d[jj, ii][:, bass.ds(q0 - 512 * jj, w)])
                        else:
                            nc.sync.dma_start(
                                out=msk[:, idx, :],
                                in_=mk_d[bass.ts(t, 128), js])
                if first_wo:
                    # after the first mask tiles so they aren't queued behind
                    # 8.4MB of wo weights
                    for dd in range(HL):
                        nc.sync.dma_start(out=wob[:, dd, :], in_=wo_d[:, dd, :])
                    first_wo = False

                # l-accumulation groups of 8 k-tiles (one TensorE broadcast
                # matmul per group; the extra DVE adds are cheap vs PE time)
                gend = {}
                tbase = 0
                while tbase < nkt:
                    glen = min(8, nkt - tbase)
                    gend[tbase + glen - 1] = (tbase, glen)
                    tbase += glen

                for h in range(HL):
                    hk = h // (HL // KVL)
                    outp = ps_o.tile([128, w], F32, tag="out")
                    lp = ps_l.tile([128, w], F32, tag="l")
                    pts = []
                    # software pipeline: PV_t is emitted one tile after QK_t so
                    # a full QK + filler sits in the PE stream while exp_t runs.
                    # Diagonal tiles contribute nothing to masked columns, so
                    # PV runs only on the live sub-range (t=0 has off=0 and
                    # start-zeroes the full width; PSUM accum is per-address).
                    def emit_pv(t):
                        off = max(0, 128 * t - q0) if mask_mode == "causal" else 0
                        nc.tensor.matmul(outp[:, off:w], V[:, t, bass.ts(hk, 128)],
                                         pts[t][:, off:w],
                                         start=(t == 0), stop=(t == nkt - 1),
                                         skip_group_check=True)

                    l_started = False
                    n_pv_done = 0
                    # narrow pieces pair adjacent k-tiles into one PSUM bank
                    # and run a single exp over the pair, halving ScalarE's
                    # per-instruction fixed overhead where it paces the tail
                    paired = w <= 256
                    t = 0
                    while t < nkt:
                        npair = 2 if (paired and t + 1 < nkt) else 1
                        stp = ps_st.tile([128, npair, w], F32, tag="st")
                        pt2 = ppool.tile([128, npair, w], BF, tag="pt")
                        for i in range(npair):
                            tt = t + i
                            # causal: columns q < 128t fully masked; compute
                            # QK only on the live sub-range.  Stale (finite)
                            # garbage in dead columns is zeroed by the mask
                            # multiply below.
                            off = (max(0, 128 * tt - q0)
                                   if mask_mode == "causal" else 0)
                            nc.tensor.matmul(stp[:, i, off:w],
                                             KT[:, hk, bass.ts(tt, 128)],
                                             QT[:, h, bass.ds(q0 + off, w - off)],
                                             start=True, stop=True)
                        if npair == 1 and mask_mode == "causal":
                            off = max(0, 128 * t - q0)
                            nc.scalar.activation(
                                out=pt2[:, 0, off:w], in_=stp[:, 0, off:w],
                                func=mybir.ActivationFunctionType.Exp)
                        else:
                            nc.scalar.activation(
                                out=pt2[:], in_=stp[:],
                                func=mybir.ActivationFunctionType.Exp)
                        for i in range(npair):
                            tt = t + i
                            if tt in atiles:
                                # multiplicative mask exp(m): 0/1 for causal;
                                # also zeroes the dead columns
                                nc.vector.tensor_mul(
                                    pt2[:, i, :], pt2[:, i, :],
                                    msk[:, atiles.index(tt), :])
                            pts.append(pt2[:, i, :])
                        # wide pieces meter po filler so backlog survives
                        # into the tapered tail (which otherwise starves and
                        # drops HAM to half clock); narrow pieces drain hard
                        po_step(3 if npair == 2 else 1)
                        # PV lags one tile behind exp so a full QK + filler
                        # sits in the PE stream while exp runs
                        while n_pv_done < len(pts) - 1:
                            emit_pv(n_pv_done)
                            n_pv_done += 1
                        t_last = t + npair - 1
                        t = t_last + 1
                        if t_last in gend:
                            # tree-reduce the group's P tiles on DVE, one
                            # broadcast l matmul per group
                            tb, glen = gend[t_last]
                            grp = pts[tb:tb + glen]
                            if glen == 1:
                                qd = grp[0]
                            else:
                                # pairwise first level, then in-place chain
                                # (bf16 adds of P<=1 values; chain depth <=3)
                                s1 = qpool.tile([128, w], BF, tag="s1")
                                nc.vector.tensor_add(s1[:], grp[0][:], grp[1][:])
                                if len(grp) >= 4:
                                    s2 = qpool.tile([128, w], BF, tag="s2")
                                    nc.vector.tensor_add(s2[:], grp[2][:],
                                                         grp[3][:])
                                    qd = qpool.tile([128, w], BF, tag="qd")
                                    nc.vector.tensor_add(qd[:], s1[:], s2[:])
                                    rest = grp[4:]
                                else:
                                    qd = qpool.tile([128, w], BF, tag="qd")
                                    if len(grp) == 3:
                                        nc.vector.tensor_add(qd[:], s1[:],
                                                             grp[2][:])
                                    else:
                                        nc.vector.tensor_copy(qd[:], s1[:])
                                    rest = []
                                for i in range(0, len(rest) - 1, 2):
                                    sp = qpool.tile([128, w], BF,
                                                    tag=f"sp{i}")
                                    nc.vector.tensor_add(sp[:], rest[i][:],
                                                         rest[i + 1][:])
                                    nc.vector.tensor_add(qd[:], qd[:], sp[:])
                                if len(rest) % 2:
                                    nc.vector.tensor_add(qd[:], qd[:],
                                                         rest[-1][:])
                            nc.tensor.matmul(lp[:], ones128[:], qd[:],
                                             start=not l_started,
                                             stop=(t_last == nkt - 1))
                            l_started = True
                    while n_pv_done < nkt:
                        emit_pv(n_pv_done)
                        n_pv_done += 1
                    # fused evacuation + normalization on DVE (ScalarE stays
                    # exp-only; 1/l is a single fast-reciprocal op on the
                    # TensorE-broadcast denominator)
                    rcp = npool.tile([128, w], F32, tag="rcp")
                    nc.vector.reciprocal_approx_fast(out=rcp[:], in_=lp[:])
                    nc.vector.tensor_mul(attnT[:, h, js], outp[:], rcp[:])
                    # PE filler between heads covers the exp pipeline refill
                    po_step(16)
                pending_po.extend(
                    (qt, nn) for qt in range(q0 // 128, (q0 + w) // 128)
                    for nn in range(D // 512))
            po_state["drain"] = True
            while pending_po or po_state["cur"] is not None:
                po_step(8)

    nc.compile()
    return nc


def _get_nc(mask_mode: str):
    if mask_mode not in _BUILD_CACHE:
        _BUILD_CACHE[mask_mode] = _build(mask_mode)
    return _BUILD_CACHE[mask_mode]


_DEINT = np.concatenate([np.arange(0, HD, 2), np.arange(1, HD, 2)])  # de-interleave


def _host_prep(x, freqs_cos, freqs_sin, mask, wq, wk, wv, wo):
    bf16 = ml_dtypes.bfloat16
    scale = float(HD) ** -0.5

    # mask mode
    mask = np.asarray(mask, np.float32)
    tril = np.tril(np.ones((S, S), bool))
    if np.all(mask == 0):
        mask_mode = "zero"
    elif np.all(mask[tril] == 0) and np.all(mask[~tril] <= -1e8):
        mask_mode = "causal"
    else:
        mask_mode = "general"

    # weights: de-interleave head dims of wq/wk; fold softmax scale into wq
    wq_p = (np.asarray(wq, np.float32).reshape(H, HD, D)[:, _DEINT, :] * scale)
    wk_p = np.asarray(wk, np.float32).reshape(KVH, HD, D)[:, _DEINT, :]
    wv_n = np.asarray(wv, np.float32).reshape(KVH, HD, D)
    wo_n = np.asarray(wo, np.float32)

    per_group = []
    for g in range(GROUPS):
        feats = np.concatenate([
            wq_p[g * HL:(g + 1) * HL].reshape(HL * HD, D),
            wk_p[g * KVL:(g + 1) * KVL].reshape(KVL * HD, D),
        ], axis=0)  # [1280, D]
        wqk_dma = np.ascontiguousarray(
            feats.reshape(FQK, 128, ND, 128).transpose(0, 3, 2, 1)).astype(bf16)
        wvg = wv_n[g * KVL:(g + 1) * KVL].reshape(KVL * HD, D)
        wv_dma = np.ascontiguousarray(
            wvg.reshape(KVL * HD, ND, 128).transpose(2, 1, 0)).astype(bf16)
        woT = wo_n[:, g * HL * HD:(g + 1) * HL * HD].T  # [1024, D]
        wo_dma = np.ascontiguousarray(
            woT.reshape(HL, 128, D).transpose(1, 0, 2)).astype(bf16)
        per_group.append((wqk_dma, wv_dma, wo_dma))

    xs = []
    for b in range(B):
        xT = np.asarray(x[b], np.float32).T  # [D, S]
        xs.append(np.ascontiguousarray(
            xT.reshape(ND, 128, S).transpose(1, 0, 2)).astype(bf16))

    cosT = np.asarray(freqs_cos, np.float32).T  # [64, S]
    sinT = np.asarray(freqs_sin, np.float32).T
    cos_dma = np.ascontiguousarray(np.concatenate([cosT, cosT], 0))
    # rotation sign folded into the sin table: o = raw*cos + halfswap(raw)*sinN
    sin_dma = np.ascontiguousarray(np.concatenate([-sinT, sinT], 0))

    # mask is applied multiplicatively after exp: P *= exp(mask)
    mask_extra = {}
    if mask_mode == "causal":
        mT = np.exp(np.minimum(mask.T, 0.0))
        md = np.empty((NJ, 4, 128, 512), np.float32)
        for j in range(NJ):
            for i in range(4):
                t = 4 * j + i
                md[j, i] = mT[t * 128:(t + 1) * 128, j * 512:(j + 1) * 512]
        mask_extra["maskd"] = md.astype(bf16)
    elif mask_mode == "general":
        with np.errstate(over="ignore"):
            mask_extra["maskt"] = np.ascontiguousarray(
                np.exp(mask.T)).astype(bf16)

    in_maps = []
    for c in range(N_CORES):
        b, g = c // GROUPS, c % GROUPS
        wqk_dma, wv_dma, wo_dma = per_group[g]
        m = {"xt": xs[b], "wqk": wqk_dma, "wv": wv_dma, "wo": wo_dma,
             "cosd": cos_dma, "sind": sin_dma}
        m.update(mask_extra)
        in_maps.append(m)
    return mask_mode, in_maps


def kernel(x, freqs_cos, freqs_sin, positions, mask, wq, wk, wv, wo,
           _want_profile=False):
    mask_mode, in_maps = _host_prep(x, freqs_cos, freqs_sin, mask, wq, wk, wv, wo)
    nc = _get_nc(mask_mode)
    res = run_bass_kernel_spmd(nc, in_maps, core_ids=list(range(N_CORES)),
                               trace=_want_profile)
    out = np.zeros((B, S, D), np.float32)
    for c in range(N_CORES):
        out[c // GROUPS] += np.asarray(res.results[c]["po"], np.float32)
    if _want_profile:
        kernel.last_exec_time_ns = res.exec_time_ns
        kernel.last_results = res
    return out


# revision 64
# speedup vs baseline: 1.1959x; 1.0033x over previous
"""Trainium2 Bass kernel for GQA attention with RoPE (nn_Attention_21603685499660).

Shapes (hardcoded): x [2, 2048, 4096], H=32 Q heads, KVH=8 KV heads, HD=128.
Sharding over 8 NeuronCores: core c -> batch b = c//4, head-group g = c%4
(8 Q heads, 2 KV heads per core).  Each core computes a partial output
(its heads' attention output through its slice of wo); the host sums the
4 partials per batch.  No on-device collectives.

Per-core pipeline (all matmuls bf16 with f32 PSUM accumulation):
  1. QKV projection from host-pre-transposed x and weights.  Q/K are
     produced directly in transposed [HD, seq] layout; V in natural
     [seq, HD] layout.  RoPE applied entirely on DVE: head dims are
     de-interleaved host-side so the rotation is a half-swap, the sign
     is folded into the prepared sin table ([-sin; +sin]), and the
     half-swap itself rides the sin-multiply via 64-partition-shifted
     access patterns (no TensorE involvement).
  2. Attention with scores computed transposed: ST[k,q] = K @ Q^T per
     (head, q piece, 128-wide k tile).  Softmax without max subtraction
     (scores are O(+-10); exp is safe in f32): P = exp(ST), applied mask
     is multiplicative (exp(mask), 0/1 for causal) on the P tile in
     bf16.  On diagonal tiles QK + exp run only on the live (unmasked)
     q sub-range.  The denominator l is quad-reduced on DVE and
     accumulated ALREADY BROADCAST on TensorE (ones128^T @ quad into a
     PSUM bank); the head output is evacuated+normalized in one DVE
     tensor_mul against a fast-reciprocal of that broadcast (no ScalarE
     Ln/Exp, so ScalarE runs exp-only with no activation-table thrash).
     Causal masks (detected host-side) skip fully-masked k tiles.  The
     q dimension is processed in pieces [512,512,512,256,256]; the
     narrow tail pieces pair adjacent k-tiles into one PSUM bank with a
     single exp per pair (halving ScalarE fixed overhead where it paces)
     and let the final piece's output projection overlap.
  3. Output projection po[q,n] += attnT[d,q]^T @ woT[d,n], emitted as
     PE filler interleaved into subsequent pieces' attention (covers
     the ACT-latency bubbles), with piece p's groups running during
     p+1; output DMAs alternate HW-DGE queues in the drain, and the
     partial outputs ship bf16 (summed f32 on host) to halve the
     33.5MB/core output traffic.

  Clock caveat: the chip's power governor sometimes latches the PE at
  2.0GHz (P0) for a whole run instead of 2.4GHz — a per-run lottery
  worth ~150us.  Denser early-phase schedules (e.g. bulk constants on
  the Act HW-DGE queue, or untapered q pieces) measured as latched far
  more often; the current schedule keeps a brief natural stall at
  ~30us and measured 747-758us across good draws.
"""

from contextlib import ExitStack

import numpy as np
import ml_dtypes

import concourse.bass as bass
import concourse.tile as tile
from concourse import bacc, mybir
from concourse.bass_utils import run_bass_kernel_spmd

B, S, D = 2, 2048, 4096
H, KVH, HD = 32, 8, 128
N_CORES = 8
GROUPS = 4            # head groups (tensor-parallel dim); B * GROUPS = 8 cores
HL = H // GROUPS      # 8 local Q heads
KVL = KVH // GROUPS   # 2 local KV heads
FQK = HL + KVL        # 10 feature tiles of 128 (Q heads then K heads)
NJ = S // 512         # 4 seq chunks of 512 (stage-1 granularity)
NT = S // 128         # 16 seq tiles of 128
ND = D // 128         # 32 contraction tiles
BF = mybir.dt.bfloat16
F32 = mybir.dt.float32

# attention q pieces (start, width); tapered tail so the last pieces'
# output projection can overlap preceding pieces
PIECES = [(0, 512), (512, 512), (1024, 512), (1536, 256), (1792, 256)]

_BUILD_CACHE: dict = {}


def _build(mask_mode: str):
    """mask_mode: 'causal' | 'zero' | 'general'."""
    nc = bacc.Bacc("TRN2", target_bir_lowering=False, debug=False,
                   num_devices=N_CORES)

    xt_d = nc.dram_tensor("xt", [128, ND, S], BF, kind="ExternalInput").ap()
    wqk_d = nc.dram_tensor("wqk", [FQK, 128, ND, 128], BF, kind="ExternalInput").ap()
    wv_d = nc.dram_tensor("wv", [128, ND, KVL * HD], BF, kind="ExternalInput").ap()
    wo_d = nc.dram_tensor("wo", [128, HL, D], BF, kind="ExternalInput").ap()
    cos_d = nc.dram_tensor("cosd", [128, S], F32, kind="ExternalInput").ap()
    sin_d = nc.dram_tensor("sind", [128, S], F32, kind="ExternalInput").ap()
    if mask_mode == "causal":
        mk_d = nc.dram_tensor("maskd", [NJ, 4, 128, 512], BF, kind="ExternalInput").ap()
    elif mask_mode == "general":
        mk_d = nc.dram_tensor("maskt", [S, S], BF, kind="ExternalInput").ap()
    # partial outputs ship bf16 (host sums in f32): halves the 33.5MB/core
    # output DMA, shrinking the drain backlog and post-matmul flush; the
    # ~0.4% partial-sum rounding is small against the 2e-2 budget
    po_d = nc.dram_tensor("po", [S, D], BF, kind="ExternalOutput").ap()

    with tile.TileContext(nc) as tc, ExitStack() as ctx:
        resident = ctx.enter_context(tc.tile_pool(name="resident", bufs=1))
        qkv = ctx.enter_context(tc.tile_pool(name="qkv", bufs=1))

        ones128 = resident.tile([128, 128], BF)
        nc.vector.memset(ones128[:], 1.0)

        QT = qkv.tile([128, HL, S], BF)    # [HD, head, seq] (de-interleaved rows)
        KT = qkv.tile([128, KVL, S], BF)
        V = qkv.tile([128, NT, KVL * HD], BF)  # [seq%128, seqtile, kv-head*HD]

        # ---- stage 1: QKV projection + RoPE ----
        with tc.tile_pool(name="s1const", bufs=1) as s1const, \
             tc.tile_pool(name="xpool", bufs=2) as xpool, \
             tc.tile_pool(name="wpool", bufs=3) as wpool, \
             tc.tile_pool(name="tpool", bufs=3) as tpool, \
             tc.tile_pool(name="ps_qk", bufs=3, space="PSUM") as ps_qk, \
             tc.tile_pool(name="ps_w", bufs=2, space="PSUM") as ps_w, \
             tc.tile_pool(name="ps_v", bufs=2, space="PSUM") as ps_v:
            cosb = s1const.tile([128, S], F32)
            sinb = s1const.tile([128, S], F32)  # [-sin; +sin] halves
            wvb = s1const.tile([128, ND, KVL * HD], BF)
            # PE warm-up: dense ones@ones matmuls (no DMA dependency) keep
            # TensorE busy through the HAM window while the first x/weight
            # DMAs land, so real matmuls start at full clock.
            for _ in range(72):
                wtile = ps_w.tile([128, 128], F32, tag="warm")
                nc.tensor.matmul(wtile[:], ones128[:], ones128[:],
                                 start=True, stop=True)

            def rope_emit(raw, f, js):
                # o = raw*cos + halfswap(raw)*sinN with no TensorE: the
                # half-swap is two partition-shifted ScalarE copies (same
                # engine as the evacuation, so ordering is free) and the
                # rotation sign lives in sinb = [-sin; +sin].
                rot = tpool.tile([128, 512], BF, tag="rot")
                nc.scalar.copy(out=rot[0:64, :], in_=raw[64:128, :])
                nc.scalar.copy(out=rot[64:128, :], in_=raw[0:64, :])
                t1 = tpool.tile([128, 512], F32, tag="t1")
                nc.vector.tensor_mul(t1[:], raw[:], cosb[:, js])
                t2 = tpool.tile([128, 512], F32, tag="t2")
                nc.vector.tensor_mul(t2[:], rot[:], sinb[:, js])
                dest = QT[:, f, js] if f < HL else KT[:, f - HL, js]
                nc.vector.tensor_add(dest, t1[:], t2[:])

            # weight prefetch runs two groups deep so the next chunk's 4MB
            # x DMA (queued between them on the same SP HW-DGE queue) never
            # head-of-line blocks the first weight tiles of the new chunk
            n_groups = NJ * FQK
            wtiles: dict = {}
            wissued = 0

            def wprefetch(upto):
                nonlocal wissued
                while wissued < min(n_groups, upto):
                    wt = wpool.tile([128, ND, 128], BF, tag="wf")
                    nc.sync.dma_start(out=wt[:], in_=wqk_d[wissued % FQK])
                    wtiles[wissued] = wt
                    wissued += 1

            wprefetch(2)
            for j in range(NJ):
                js = bass.ts(j, 512)
                xj = xpool.tile([128, ND, 512], BF)
                for n in range(ND):
                    nc.sync.dma_start(out=xj[:, n, :], in_=xt_d[:, n, js])
                for f in range(FQK):
                    gi = j * FQK + f
                    wf = wtiles.pop(gi)
                    wprefetch(gi + 3)
                    if f == 0:
                        # cos/sin loaded per chunk (0.5MB slices) and wv
                        # deferred to mid-chunk-0: keeps the SP queue free
                        # of multi-MB head-of-line insertions (all stay on
                        # the SP queue — moving them to the Act queue
                        # measured 937us, likely P0-governor related)
                        nc.sync.dma_start(out=cosb[:, js], in_=cos_d[:, js])
                        nc.sync.dma_start(out=sinb[:, js], in_=sin_d[:, js])
                    if j == 0 and f == 8:
                        nc.sync.dma_start(out=wvb[:], in_=wv_d[:])
                    ps = ps_qk.tile([128, 512], F32, tag="qk")
                    for n in range(ND):
                        nc.tensor.matmul(ps[:], wf[:, n, :], xj[:, n, :],
                                         start=(n == 0), stop=(n == ND - 1))
                    raw = tpool.tile([128, 512], BF, tag="raw")
                    nc.scalar.copy(out=raw[:], in_=ps[:])
                    rope_emit(raw, f, js)
                for tt in range(4):
                    psv = ps_v.tile([128, KVL * HD], F32, tag="v")
                    for n in range(ND):
                        nc.tensor.matmul(psv[:], xj[:, n, bass.ts(tt, 128)],
                                         wvb[:, n, :],
                                         start=(n == 0), stop=(n == ND - 1))
                    nc.scalar.copy(out=V[:, j * 4 + tt, :], in_=psv[:])

        # attnT + wo live from stage 2 through stage 3 (pool opened only now
        # so stage 1 had the SBUF).
        att_out = ctx.enter_context(tc.tile_pool(name="att_out", bufs=1))
        attnT = att_out.tile([128, HL, S], BF)  # [HD, head, seq]
        wob = att_out.tile([128, HL, D], BF)

        # ---- stage 2+3: attention with interleaved output projection ----
        # Per (piece, h) k-loop: QK -> exp -> (0/1 mask multiply in bf16
        # SBUF) -> [l, PV] where the softmax denominator accumulates
        # broadcast on TensorE (ones128^T @ quad into a PSUM bank) so no
        # serial DVE chain gates the pipeline; 1/l is one fast-reciprocal
        # DVE op.  The raw output is evacuated and normalized on DVE.
        # Output-projection (po) matmul groups for piece p are emitted
        # during later pieces' head loops — dense PE filler for the
        # ACT-bound attention stretches.
        po_state = {"cur": None, "dd": 0, "drain": False, "alt": False}

        def po_step(budget):
            # emit up to `budget` output-projection matmuls as PE filler;
            # a group's PSUM accumulation legally interleaves with other
            # banks' matmuls, so groups can be spread across many call sites
            for _ in range(budget):
                if po_state["cur"] is None:
                    if not pending_po:
                        return
                    qt, nn = pending_po.pop(0)
                    pop = ps_po.tile([128, 512], F32, tag="po")
                    po_state["cur"] = (qt, nn, pop)
                    po_state["dd"] = 0
                qt, nn, pop = po_state["cur"]
                dd = po_state["dd"]
                nc.tensor.matmul(pop[:], attnT[:, dd, bass.ts(qt, 128)],
                                 wob[:, dd, bass.ts(nn, 512)],
                                 start=(dd == 0), stop=(dd == HL - 1))
                po_state["dd"] += 1
                if po_state["dd"] == HL:
                    stg = spool.tile([128, 512], BF, tag="stg")
                    nc.vector.tensor_copy(stg[:], pop[:])
                    # in the final drain ScalarE is idle: alternate output
                    # DMAs onto its HW-DGE queue so the write backlog
                    # doesn't tail the last matmuls on a single queue
                    eng = nc.scalar if (po_state["drain"] and po_state["alt"]) \
                        else nc.sync
                    po_state["alt"] = not po_state["alt"]
                    eng.dma_start(
                        out=po_d[bass.ts(qt, 128), bass.ts(nn, 512)], in_=stg[:])
                    po_state["cur"] = None

        with tc.tile_pool(name="mpool", bufs=2 if mask_mode != "general" else 1) as mpool, \
             tc.tile_pool(name="ppool", bufs=12) as ppool, \
             tc.tile_pool(name="qpool", bufs=2) as qpool, \
             tc.tile_pool(name="npool", bufs=2) as npool, \
             tc.tile_pool(name="spool", bufs=3) as spool, \
             tc.tile_pool(name="ps_st", bufs=3, space="PSUM") as ps_st, \
             tc.tile_pool(name="ps_o", bufs=1, space="PSUM") as ps_o, \
             tc.tile_pool(name="ps_l", bufs=1, space="PSUM") as ps_l, \
             tc.tile_pool(name="ps_po", bufs=3, space="PSUM") as ps_po:
            pending_po = []  # (qt, nn) groups ready to emit as PE filler
            first_wo = True
            for pidx, (q0, w) in enumerate(PIECES):
                if pidx == len(PIECES) - 1:
                    po_state["drain"] = True
                js = bass.ds(q0, w)
                if mask_mode == "zero":
                    nkt = NT
                    atiles = []
                else:
                    nkt = (q0 + w) // 128 if mask_mode == "causal" else NT
                    if mask_mode == "causal":
                        atiles = list(range(q0 // 128, (q0 + w) // 128))
                    else:
                        atiles = list(range(nkt))
                if atiles:
                    msk = mpool.tile([128, len(atiles), w], BF, tag="msk")
                    for idx, t in enumerate(atiles):
                        if mask_mode == "causal":
                            jj, ii = t // 4, t % 4
                            nc.sync.dma_start(
                                out=msk[:, idx, :],
                                in_=mk_d[jj, ii][:, bass.ds(q0 - 512 * jj, w)])
                        else:
                            nc.sync.dma_start(
                                out=msk[:, idx, :],
                                in_=mk_d[bass.ts(t, 128), js])
                if first_wo:
                    # after the first mask tiles so they aren't queued behind
                    # 8.4MB of wo weights
                    for dd in range(HL):
                        nc.sync.dma_start(out=wob[:, dd, :], in_=wo_d[:, dd, :])
                    first_wo = False

                # l-accumulation groups of 8 k-tiles (one TensorE broadcast
                # matmul per group; the extra DVE adds are cheap vs PE time)
                gend = {}
                tbase = 0
                while tbase < nkt:
                    glen = min(8, nkt - tbase)
                    gend[tbase + glen - 1] = (tbase, glen)
                    tbase += glen

                for h in range(HL):
                    hk = h // (HL // KVL)
                    outp = ps_o.tile([128, w], F32, tag="out")
                    lp = ps_l.tile([128, w], F32, tag="l")
                    pts = []
                    # software pipeline: PV_t is emitted one tile after QK_t so
                    # a full QK + filler sits in the PE stream while exp_t runs.
                    # Diagonal tiles contribute nothing to masked columns, so
                    # PV runs only on the live sub-range (t=0 has off=0 and
                    # start-zeroes the full width; PSUM accum is per-address).
                    def emit_pv(t):
                        off = max(0, 128 * t - q0) if mask_mode == "causal" else 0
                        nc.tensor.matmul(outp[:, off:w], V[:, t, bass.ts(hk, 128)],
                                         pts[t][:, off:w],
                                         start=(t == 0), stop=(t == nkt - 1),
                                         skip_group_check=True)

                    l_started = False
                    n_pv_done = 0
                    # narrow pieces pair adjacent k-tiles into one PSUM bank
                    # and run a single exp over the pair, halving ScalarE's
                    # per-instruction fixed overhead where it paces the tail
                    paired = w <= 256
                    t = 0
                    while t < nkt:
                        npair = 2 if (paired and t + 1 < nkt) else 1
                        stp = ps_st.tile([128, npair, w], F32, tag="st")
                        pt2 = ppool.tile([128, npair, w], BF, tag="pt")
                        for i in range(npair):
                            tt = t + i
                            # causal: columns q < 128t fully masked; compute
                            # QK only on the live sub-range.  Stale (finite)
                            # garbage in dead columns is zeroed by the mask
                            # multiply below.
                            off = (max(0, 128 * tt - q0)
                                   if mask_mode == "causal" else 0)
                            nc.tensor.matmul(stp[:, i, off:w],
                                             KT[:, hk, bass.ts(tt, 128)],
                                             QT[:, h, bass.ds(q0 + off, w - off)],
                                             start=True, stop=True)
                        if npair == 1 and mask_mode == "causal":
                            off = max(0, 128 * t - q0)
                            nc.scalar.activation(
                                out=pt2[:, 0, off:w], in_=stp[:, 0, off:w],
                                func=mybir.ActivationFunctionType.Exp)
                        else:
                            nc.scalar.activation(
                                out=pt2[:], in_=stp[:],
                                func=mybir.ActivationFunctionType.Exp)
                        for i in range(npair):
                            tt = t + i
                            if tt in atiles:
                                # multiplicative mask exp(m): 0/1 for causal;
                                # also zeroes the dead columns
                                nc.vector.tensor_mul(
                                    pt2[:, i, :], pt2[:, i, :],
                                    msk[:, atiles.index(tt), :])
                            pts.append(pt2[:, i, :])
                        # wide pieces meter po filler so backlog survives
                        # into the tapered tail (which otherwise starves and
                        # drops HAM to half clock); narrow pieces drain hard
                        po_step(3 if npair == 2 else 1)
                        # PV lags one tile behind exp so a full QK + filler
                        # sits in the PE stream while exp runs
                        while n_pv_done < len(pts) - 1:
                            emit_pv(n_pv_done)
                            n_pv_done += 1
                        t_last = t + npair - 1
                        t = t_last + 1
                        if t_last in gend:
                            # tree-reduce the group's P tiles on DVE, one
                            # broadcast l matmul per group
                            tb, glen = gend[t_last]
                            grp = pts[tb:tb + glen]
                            if glen == 1:
                                qd = grp[0]
                            else:
                                # pairwise first level, then in-place chain
                                # (bf16 adds of P<=1 values; chain depth <=3)
                                s1 = qpool.tile([128, w], BF, tag="s1")
                                nc.vector.tensor_add(s1[:], grp[0][:], grp[1][:])
                                if len(grp) >= 4:
                                    s2 = qpool.tile([128, w], BF, tag="s2")
                                    nc.vector.tensor_add(s2[:], grp[2][:],
                                                         grp[3][:])
                                    qd = qpool.tile([128, w], BF, tag="qd")
                                    nc.vector.tensor_add(qd[:], s1[:], s2[:])
                                    rest = grp[4:]
                                else:
                                    qd = qpool.tile([128, w], BF, tag="qd")
                                    if len(grp) == 3:
                                        nc.vector.tensor_add(qd[:], s1[:],
                                                             grp[2][:])
                                    else:
                                        nc.vector.tensor_copy(qd[:], s1[:])
                                    rest = []
                                for i in range(0, len(rest) - 1, 2):
                                    sp = qpool.tile([128, w], BF,
                                                    tag=f"sp{i}")
                                    nc.vector.tensor_add(sp[:], rest[i][:],
                                                         rest[i + 1][:])
                                    nc.vector.tensor_add(qd[:], qd[:], sp[:])
                                if len(rest) % 2:
                                    nc.vector.tensor_add(qd[:], qd[:],
                                                         rest[-1][:])
                            nc.tensor.matmul(lp[:], ones128[:], qd[:],
                                             start=not l_started,
                                             stop=(t_last == nkt - 1))
                            l_started = True
                    while n_pv_done < nkt:
                        emit_pv(n_pv_done)
                        n_pv_done += 1
                    # fused evacuation + normalization on DVE (ScalarE stays
                    # exp-only; 1/l is a single fast-reciprocal op on the
                    # TensorE-broadcast denominator)
                    rcp = npool.tile([128, w], F32, tag="rcp")
                    nc.vector.reciprocal_approx_fast(out=rcp[:], in_=lp[:])
                    nc.vector.tensor_mul(attnT[:, h, js], outp[:], rcp[:])
                    # PE filler between heads covers the exp pipeline refill
                    po_step(16)
                pending_po.extend(
                    (qt, nn) for qt in range(q0 // 128, (q0 + w) // 128)
                    for nn in range(D // 512))
            po_state["drain"] = True
            while pending_po or po_state["cur"] is not None:
                po_step(8)

    nc.compile()
    return nc


def _get_nc(mask_mode: str):
    if mask_mode not in _BUILD_CACHE:
        _BUILD_CACHE[mask_mode] = _build(mask_mode)
    return _BUILD_CACHE[mask_mode]


_DEINT = np.concatenate([np.arange(0, HD, 2), np.arange(1, HD, 2)])  # de-interleave


def _host_prep(x, freqs_cos, freqs_sin, mask, wq, wk, wv, wo):
    bf16 = ml_dtypes.bfloat16
    scale = float(HD) ** -0.5

    # mask mode
    mask = np.asarray(mask, np.float32)
    tril = np.tril(np.ones((S, S), bool))
    if np.all(mask == 0):
        mask_mode = "zero"
    elif np.all(mask[tril] == 0) and np.all(mask[~tril] <= -1e8):
        mask_mode = "causal"
    else:
        mask_mode = "general"

    # weights: de-interleave head dims of wq/wk; fold softmax scale into wq
    wq_p = (np.asarray(wq, np.float32).reshape(H, HD, D)[:, _DEINT, :] * scale)
    wk_p = np.asarray(wk, np.float32).reshape(KVH, HD, D)[:, _DEINT, :]
    wv_n = np.asarray(wv, np.float32).reshape(KVH, HD, D)
    wo_n = np.asarray(wo, np.float32)

    per_group = []
    for g in range(GROUPS):
        feats = np.concatenate([
            wq_p[g * HL:(g + 1) * HL].reshape(HL * HD, D),
            wk_p[g * KVL:(g + 1) * KVL].reshape(KVL * HD, D),
        ], axis=0)  # [1280, D]
        wqk_dma = np.ascontiguousarray(
            feats.reshape(FQK, 128, ND, 128).transpose(0, 3, 2, 1)).astype(bf16)
        wvg = wv_n[g * KVL:(g + 1) * KVL].reshape(KVL * HD, D)
        wv_dma = np.ascontiguousarray(
            wvg.reshape(KVL * HD, ND, 128).transpose(2, 1, 0)).astype(bf16)
        woT = wo_n[:, g * HL * HD:(g + 1) * HL * HD].T  # [1024, D]
        wo_dma = np.ascontiguousarray(
            woT.reshape(HL, 128, D).transpose(1, 0, 2)).astype(bf16)
        per_group.append((wqk_dma, wv_dma, wo_dma))

    xs = []
    for b in range(B):
        xT = np.asarray(x[b], np.float32).T  # [D, S]
        xs.append(np.ascontiguousarray(
            xT.reshape(ND, 128, S).transpose(1, 0, 2)).astype(bf16))

    cosT = np.asarray(freqs_cos, np.float32).T  # [64, S]
    sinT = np.asarray(freqs_sin, np.float32).T
    cos_dma = np.ascontiguousarray(np.concatenate([cosT, cosT], 0))
    # rotation sign folded into the sin table: o = raw*cos + halfswap(raw)*sinN
    sin_dma = np.ascontiguousarray(np.concatenate([-sinT, sinT], 0))

    # mask is applied multiplicatively after exp: P *= exp(mask)
    mask_extra = {}
    if mask_mode == "causal":
        mT = np.exp(np.minimum(mask.T, 0.0))
        md = np.empty((NJ, 4, 128, 512), np.float32)
        for j in range(NJ):
            for i in range(4):
                t = 4 * j + i
                md[j, i] = mT[t * 128:(t + 1) * 128, j * 512:(j + 1) * 512]
        mask_extra["maskd"] = md.astype(bf16)
    elif mask_mode == "general":
        with np.errstate(over="ignore"):
            mask_extra["maskt"] = np.ascontiguousarray(
                np.exp(mask.T)).astype(bf16)

    in_maps = []
    for c in range(N_CORES):
        b, g = c // GROUPS, c % GROUPS
        wqk_dma, wv_dma, wo_dma = per_group[g]
        m = {"xt": xs[b], "wqk": wqk_dma, "wv": wv_dma, "wo": wo_dma,
             "cosd": cos_dma, "sind": sin_dma}
        m.update(mask_extra)
        in_maps.append(m)
    return mask_mode, in_maps


def kernel(x, freqs_cos, freqs_sin, positions, mask, wq, wk, wv, wo,
           _want_profile=False):
    mask_mode, in_maps = _host_prep(x, freqs_cos, freqs_sin, mask, wq, wk, wv, wo)
    nc = _get_nc(mask_mode)
    res = run_bass_kernel_spmd(nc, in_maps, core_ids=list(range(N_CORES)),
                               trace=_want_profile)
    out = np.zeros((B, S, D), np.float32)
    for c in range(N_CORES):
        out[c // GROUPS] += np.asarray(res.results[c]["po"], np.float32)
    if _want_profile:
        kernel.last_exec_time_ns = res.exec_time_ns
        kernel.last_results = res
    return out
